# revision 1
# baseline (speedup 1.0000x reference)
import sys
from contextlib import ExitStack

import numpy as np

sys.path.insert(0, "/opt/trn_rl_repo")

import concourse.bass as bass  # noqa: E402
import concourse.mybir as mybir  # noqa: E402
import concourse.tile as tile  # noqa: E402
from concourse import bacc  # noqa: E402
from concourse.bass_utils import run_bass_kernel_spmd  # noqa: E402

C = 64
N_CORES = 8

# Pair table: each entry computes taps (ka, kb) of the 3x3 dynamic filter in
# one [K=64, M=128] matmul (top 64 psum partitions = ka's channels, bottom 64
# = kb's channels).  `tl` selects which staged x-tile supplies the shifted
# patch operand: tile 'A' has its bottom half pre-shifted by +1 element
# (delta (0,1)), tile 'B' by +S elements (delta (1,0)).  (i, j) is the
# padded-layout offset of tap ka; tap kb's shift comes from the tile's
# pre-shifted bottom content.  Tap k=5 appears in both B-pairs with halved
# weights/bias so the products sum to the correct single contribution.
PAIRS = [
    (0, 1, "A", 0, 0),
    (3, 4, "A", 1, 0),
    (6, 7, "A", 2, 0),
    (2, 5, "B", 0, 2),
    (5, 8, "B", 1, 2),
]

F16 = mybir.dt.float16
F32 = mybir.dt.float32


def pack_weights(w_gen: np.ndarray, b_gen: np.ndarray):
    """Host-side packing of the tiny 1x1 generator weights.

    Returns
      wt   [64, 5*128] fp16 : lhsT blocks; block j col (half*64+c) holds
                              W[c, k, :] (tap k of pair j, half-scaled for k=5)
      bias [128, 5]    fp32 : per-partition bias for pair j
      idt  [128, 64]   fp16 : stacked identity [I64; I64] for the fold matmul
    """
    W = w_gen.reshape(C, 9, C).astype(np.float32)  # [c, k, c']
    b = b_gen.reshape(C, 9).astype(np.float32)
    wt = np.zeros((C, 5 * 128), np.float32)
    bias = np.zeros((128, 5), np.float32)
    for jdx, (ka, kb, _, _, _) in enumerate(PAIRS):
        for half, k in ((0, ka), (1, kb)):
            s = 0.5 if k == 5 else 1.0
            wt[:, jdx * 128 + half * 64 : jdx * 128 + half * 64 + C] = W[:, k, :].T * s
            bias[half * 64 : half * 64 + C, jdx] = b[:, k] * s
    idt = np.concatenate([np.eye(C), np.eye(C)], axis=0)
    # duplicate the lhsT rows so PE row-group B (partitions 64-127) can read
    # its stationary from the matching partition range
    wt2 = np.concatenate([wt, wt], axis=0)
    return wt2.astype(np.float16), bias.astype(np.float32), idt.astype(np.float16)


def build_nc(H=128, W=128, CH=4):
    """Build the single-core Bass program (SPMD across cores).

    H, W: spatial dims; CH: image rows per chunk.
    """
    S = W + 2  # padded row stride
    PS = (H + 2) * S  # padded plane size
    Nc = CH * W  # pixels per chunk
    nch = H // CH
    mm_cols = min(512, Nc)  # psum-bank limit for fp32 matmul output
    assert Nc % mm_cols == 0
    rpm = mm_cols // W  # image rows per matmul
    assert rpm * W == mm_cols

    nc = bacc.Bacc("TRN2", target_bir_lowering=False)
    x_in = nc.declare_dram_parameter("x", [C, H, W], F32, isOutput=False)
    wt_in = nc.declare_dram_parameter("wt", [128, 5 * 128], F16, isOutput=False)
    bias_in = nc.declare_dram_parameter("bias", [128, 5], F32, isOutput=False)
    idt_in = nc.declare_dram_parameter("idt", [128, C], F16, isOutput=False)
    out_ext = nc.declare_dram_parameter("out", [C, H, W], F32, isOutput=True)

    add = mybir.AluOpType.add
    mult = mybir.AluOpType.mult
    Identity = mybir.ActivationFunctionType.Identity

    with ExitStack() as ctx:
        tc = ctx.enter_context(tile.TileContext(nc))
        const = ctx.enter_context(tc.tile_pool(name="const", bufs=1))
        fpsum = ctx.enter_context(tc.tile_pool(name="fpsum", bufs=6, space="PSUM"))
        opsum = ctx.enter_context(tc.tile_pool(name="opsum", bufs=1, space="PSUM"))
        prod = ctx.enter_context(tc.tile_pool(name="prod", bufs=16))
        outp = ctx.enter_context(tc.tile_pool(name="outp", bufs=6))

        XA = const.tile([128, PS], F16)
        XB = const.tile([128, PS], F16)
        WT = const.tile([128, 5 * 128], F16)
        BIAS = const.tile([128, 5], F32)
        IDT = const.tile([128, C], F16)

        nc.sync.dma_start(WT[:], wt_in[:])
        nc.sync.dma_start(BIAS[:], bias_in[:])
        nc.sync.dma_start(IDT[:], idt_in[:])

        xa3 = XA[:].rearrange("p (h w) -> p h w", h=H + 2)
        xb3 = XB[:].rearrange("p (h w) -> p h w", h=H + 2)

        # zero only the padding borders of the top plane (bottoms/XB inherit
        # them through the staged copies)
        nc.vector.memset(xa3[0:C, 0, :], 0.0)
        nc.vector.memset(xa3[0:C, H + 1, :], 0.0)
        nc.gpsimd.memset(xa3[0:C, 1 : H + 1, 0], 0.0)
        nc.gpsimd.memset(xa3[0:C, 1 : H + 1, W + 1], 0.0)

        # piecewise: GPSIMD cast-DMA loads a band of x (fp32 -> fp16) into the
        # padded interior, then HWDGE makes the shifted/replicated copies.
        # First bands are small so the main loop starts early.
        row_bands = sorted({min(r, H) for r in [0, 4, 8, 16, 24]} | set(range(24, H + 1, 16)) | {H})
        for b in range(len(row_bands) - 1):
            r0, r1 = row_bands[b], row_bands[b + 1]
            nc.gpsimd.dma_start(
                out=xa3[0:C, 1 + r0 : 1 + r1, 1 : W + 1],
                in_=x_in[:, r0:r1, :],
            )
            # staged copies, clipped to the shifted-content range
            lo = r0 * S
            hi = r1 * S if r1 < H else PS
            ha = min(hi, PS - 1)
            hb = min(hi, PS - S)
            nc.sync.dma_start(out=XA[C:128, lo:ha], in_=XA[0:C, lo + 1 : ha + 1])
            nc.sync.dma_start(out=XB[0:C, lo:hi], in_=XA[0:C, lo:hi])
            nc.sync.dma_start(out=XB[C:128, lo:hb], in_=XA[0:C, lo + S : hb + S])

        N_EXTRACT = 3  # f-tiles extracted by ScalarE (bias fused there)
        # row-group assignment: pairs 0-2 stream through PE rows 0-63 (rhs =
        # XA top = x), pairs 3-4 through rows 64-127 (rhs = XC bottom = x) --
        # the two K=64 streams run concurrently on disjoint row-groups.
        GROUP_B = (3, 4)

        EMIT_ORDER = [0, 3, 1, 4, 2]  # alternate A/B row-groups on the PE

        def trio_window(tile_ap, off, count):
            """[128, count, CH, W] sliding window over the padded plane:
            outer dim and row dim share the +S stride (overlapping reads)."""
            base = tile_ap[:, off : off + 1]
            w = base.copy()
            w.ap = mybir.VecI64Pair(
                [tuple(w.ap[0]), (S, count), (S, CH), (1, W)]
            )
            return w

        op = None
        for n in range(nch):
            h0 = n * CH
            Ps = [None] * len(PAIRS)
            # FB/PT hold the three ScalarE-extracted pair-tiles side by side;
            # their patch windows are +130 apart, so a single 4D-AP DVE
            # multiply covers all three products at the 2x fp16 rate.
            FB = prod.tile([128, N_EXTRACT * Nc], F16, tag="fb")
            PT = prod.tile([128, N_EXTRACT * Nc], F16, tag="pt")
            for jdx in EMIT_ORDER:
                ka, kb, tl, i, j = PAIRS[jdx]
                fp = fpsum.tile([128, Nc], F32, tag="fp")
                grp_b = jdx in GROUP_B
                for m in range(Nc // mm_cols):
                    r0 = h0 + 1 + m * rpm
                    if grp_b:
                        lhsT = WT[C:128, jdx * 128 : (jdx + 1) * 128]
                        rhs = xb3[C:128, r0 - 1 : r0 - 1 + rpm, 1 : W + 1]
                        tpos = (64, 0)
                    else:
                        lhsT = WT[0:C, jdx * 128 : (jdx + 1) * 128]
                        rhs = xa3[0:C, r0 : r0 + rpm, 1 : W + 1]
                        tpos = (0, 0)
                    nc.tensor.matmul(
                        fp[:, m * mm_cols : (m + 1) * mm_cols],
                        lhsT,
                        rhs,
                        start=True,
                        stop=True,
                        tile_position=tpos,
                    )
                if jdx < N_EXTRACT:
                    # ScalarE evacuates f (+bias) to SBUF fp16
                    nc.scalar.activation(
                        FB[:, jdx * Nc : (jdx + 1) * Nc],
                        fp[:],
                        Identity,
                        bias=BIAS[:, jdx : jdx + 1],
                    )
                else:
                    # DVE reads f straight from PSUM (1x) with bias fused
                    src3 = xa3 if tl == "A" else xb3
                    in1 = src3[:, h0 + i : h0 + i + CH, j : j + W]
                    P = prod.tile([128, Nc], F16, tag="p")
                    P3 = P[:].rearrange("p (a b) -> p a b", a=CH)
                    fp3 = fp[:].rearrange("p (a b) -> p a b", a=CH)
                    nc.vector.scalar_tensor_tensor(
                        P3, fp3, BIAS[:, jdx : jdx + 1], in1, add, mult
                    )
                    Ps[jdx] = P[:]
            FB4 = FB[:].rearrange("p (e a b) -> p e a b", e=N_EXTRACT, a=CH)
            PT4 = PT[:].rearrange("p (e a b) -> p e a b", e=N_EXTRACT, a=CH)
            nc.vector.tensor_tensor(
                PT4, FB4, trio_window(XA[:], h0 * S, N_EXTRACT), mult
            )
            for jdx in range(N_EXTRACT):
                Ps[jdx] = PT[:, jdx * Nc : (jdx + 1) * Nc]

            # fold pair halves and accumulate into the output psum; grouped
            # at chunk end so the identity stationary loads once per chunk
            # two chunks accumulate into one double-width output psum so
            # the psum->sbuf copy and the store DMA run at half the op count
            if n % 2 == 0:
                op = opsum.tile([C, 2 * Nc], F32, tag="op")
            base = (n % 2) * Nc
            for fi, jdx in enumerate(range(len(PAIRS))):
                P = Ps[jdx]
                for m in range(Nc // mm_cols):
                    sl = slice(m * mm_cols, (m + 1) * mm_cols)
                    osl = slice(base + m * mm_cols, base + (m + 1) * mm_cols)
                    nc.tensor.matmul(
                        op[:, osl],
                        IDT[:],
                        P[:, sl],
                        start=(fi == 0),
                        stop=(fi == len(PAIRS) - 1),
                    )

            if n % 2 == 1:
                OUT = outp.tile([C, 2 * Nc], F32)
                nc.scalar.copy(OUT[:], op[:])
                o3 = OUT[:].rearrange("p (a b) -> p a b", a=2 * CH)
                nc.sync.dma_start(out_ext[:, h0 - CH : h0 + CH, :], o3)

    nc.compile()
    return nc


_NC_CACHE = {}


def _get_nc(H, W, CH):
    key = (H, W, CH)
    if key not in _NC_CACHE:
        _NC_CACHE[key] = build_nc(H, W, CH)
    return _NC_CACHE[key]


def run(x, w_gen, b_gen, trace=False, tmpdir=None):
    x = np.asarray(x, dtype=np.float32)
    w_gen = np.asarray(w_gen, dtype=np.float32)
    b_gen = np.asarray(b_gen, dtype=np.float32)
    B, c, H, W = x.shape
    assert c == C and B == N_CORES

    wt, bias, idt = pack_weights(w_gen, b_gen)
    nc = _get_nc(H, W, 4)

    in_maps = [
        {"x": np.ascontiguousarray(x[i]), "wt": wt, "bias": bias, "idt": idt}
        for i in range(B)
    ]
    res = run_bass_kernel_spmd(
        nc, in_maps, core_ids=list(range(N_CORES)), trace=trace, tmpdir=tmpdir
    )
    out = np.stack([res.results[i]["out"] for i in range(B)], axis=0)
    return out, res


def kernel(x: np.ndarray, w_gen: np.ndarray, b_gen: np.ndarray) -> np.ndarray:
    return run(x, w_gen, b_gen)[0]



# revision 9
# speedup vs baseline: 1.1433x; 1.1433x over previous
import sys
from contextlib import ExitStack

import numpy as np

sys.path.insert(0, "/opt/trn_rl_repo")

import concourse.bass as bass  # noqa: E402
import concourse.mybir as mybir  # noqa: E402
import concourse.tile as tile  # noqa: E402
from concourse import bacc  # noqa: E402
from concourse.bass_utils import run_bass_kernel_spmd  # noqa: E402

C = 64
N_CORES = 8

F16 = mybir.dt.float16
F32 = mybir.dt.float32

# Half-image layout: partitions 0:64 = channels of image rows 0..63,
# partitions 64:128 = channels of rows 64..127.  A single padded fp16 plane
# [128, (H/2+2)*(W+2)] serves every tap as a plain offset view (identical
# offset for both halves), so no pre-shifted staging copies are needed and
# the output needs no cross-partition fold.
#
# Per tap k=(di,dj): f_k = blockdiag(Wk^T, Wk^T) @ x  (K=128 matmul, both
# halves at once), then T_k = (f_k + b_k) * patch_k elementwise, and
# out = sum_k T_k.
#
# Product routing (per 9 taps):
#   EXT  (dj=0 trio + dj=2 pair): ScalarE extracts f+b to fp16, DVE
#        multiplies 3/2 taps per op via 4D strided window APs (2x fp16 rate)
#   SDVE (tap 8): DVE scalar_tensor_tensor straight from PSUM
#   POOL (dj=1 trio): GpSimd scalar_tensor_tensor straight from PSUM
# Fold: DVE pairwise adds reduce the 5 extracted products to one tile;
# PE identity matmuls accumulate that tile + the 4 PSUM-route tiles into
# the output psum, which ScalarE copies out for the store DMA.

EXT = {0: 0, 3: 1, 6: 2, 2: 3, 5: 4, 8: 5}  # tap -> FB slot; dj=0 and dj=2 trios
SDVE = [1, 4, 7]  # dj=1 trio: DVE STT straight from PSUM (GpSimd can't read PSUM)


def pack_weights(w_gen: np.ndarray, b_gen: np.ndarray):
    W3 = w_gen.reshape(C, 9, C).astype(np.float32)  # [c, k, c']
    b3 = b_gen.reshape(C, 9).astype(np.float32)
    wt = np.zeros((128, 9 * 128), np.float32)
    bias = np.zeros((128, 9), np.float32)
    for k in range(9):
        blk = W3[:, k, :].T  # [c', c]
        wt[0:C, k * 128 : k * 128 + C] = blk
        wt[C:128, k * 128 + C : k * 128 + 128] = blk
        bias[0:C, k] = b3[:, k]
        bias[C:128, k] = b3[:, k]
    idt = np.eye(128, dtype=np.float32)
    return wt.astype(np.float16), bias.astype(np.float32), idt.astype(np.float16)


def build_nc(H=128, W=128, CH=8):
    HH = H // 2  # rows per half
    S = W + 2  # padded row stride
    PR = HH + 2  # padded rows per half-plane
    PS = PR * S
    Nc = CH * W  # pixels per chunk (per half)
    nch = HH // CH
    mm_cols = 512  # psum-bank limit for fp32 matmul output
    rpm = mm_cols // W
    nmm = Nc // mm_cols

    nc = bacc.Bacc("TRN2", target_bir_lowering=False)
    x_in = nc.declare_dram_parameter("x", [C, H, W], F32, isOutput=False)
    wt_in = nc.declare_dram_parameter("wt", [128, 9 * 128], F16, isOutput=False)
    bias_in = nc.declare_dram_parameter("bias", [128, 9], F32, isOutput=False)
    idt_in = nc.declare_dram_parameter("idt", [128, 128], F16, isOutput=False)
    out_ext = nc.declare_dram_parameter("out", [C, H, W], F32, isOutput=True)

    add = mybir.AluOpType.add
    mult = mybir.AluOpType.mult
    Identity = mybir.ActivationFunctionType.Identity

    with ExitStack() as ctx:
        tc = ctx.enter_context(tile.TileContext(nc))
        const = ctx.enter_context(tc.tile_pool(name="const", bufs=1))
        fpsum = ctx.enter_context(tc.tile_pool(name="fpsum", bufs=3, space="PSUM"))
        opsum = ctx.enter_context(tc.tile_pool(name="opsum", bufs=1, space="PSUM"))
        fbp = ctx.enter_context(tc.tile_pool(name="fbp", bufs=2))
        ptp = ctx.enter_context(tc.tile_pool(name="ptp", bufs=2))
        pp = ctx.enter_context(tc.tile_pool(name="pp", bufs=6))
        dp = ctx.enter_context(tc.tile_pool(name="dp", bufs=4))
        outp = ctx.enter_context(tc.tile_pool(name="outp", bufs=2))

        X = const.tile([128, PS], F16)
        WT = const.tile([128, 9 * 128], F16)
        BIAS = const.tile([128, 9], F32)
        IDT = const.tile([128, 128], F16)

        nc.sync.dma_start(WT[:], wt_in[:])
        nc.sync.dma_start(BIAS[:], bias_in[:])
        nc.sync.dma_start(IDT[:], idt_in[:])

        x3 = X[:].rearrange("p (h w) -> p h w", h=PR)

        # zero the borders: top halo row of the top half, bottom halo row of
        # the bottom half, and the left/right pad columns everywhere
        nc.vector.memset(x3[0:C, 0, :], 0.0)
        nc.vector.memset(x3[C:128, PR - 1, :], 0.0)
        nc.gpsimd.memset(x3[:, :, 0], 0.0)
        nc.gpsimd.memset(x3[:, :, W + 1], 0.0)

        # banded cast-loads (fp32 -> fp16): plane rows 1..65 of the top half
        # are image rows 0..64; plane rows 0..64 of the bottom half are image
        # rows 63..127.  Early bands are small so chunk 0 starts quickly.
        bands = [0, CH + 2] + [CH + 2 + CH * i for i in range(1, nch - 1)] + [PR - 1]
        for b in range(len(bands) - 1):
            r0, r1 = bands[b], bands[b + 1]
            nc.gpsimd.dma_start(
                out=x3[0:C, 1 + r0 : 1 + r1, 1 : W + 1],
                in_=x_in[:, r0:r1, :],
            )
            nc.gpsimd.dma_start(
                out=x3[C:128, r0:r1, 1 : W + 1],
                in_=x_in[:, HH - 1 + r0 : HH - 1 + r1, :],
            )

        def win(base_row, dj, count, ch):
            """[128, count, ch, W] window ([128, ch, W] when count is None):
            tap dim and row dim share the +S stride (overlapping reads)."""
            base = x3[:, base_row : base_row + 1, dj : dj + 1]
            w = base.copy()
            dims = [tuple(w.ap[0])]
            if count is not None:
                dims.append((S, count))
            dims += [(S, ch), (1, W)]
            w.ap = mybir.VecI64Pair(dims)
            return w

        prev_fold = None  # (tiles, r0) from the previous chunk
        for n in range(nch):
            r0 = n * CH
            FB = fbp.tile([128, 6 * Nc], F16, tag="fb")
            FB4 = FB[:].rearrange("p (e a b) -> p e a b", e=6, a=CH)
            Tp = [None] * 9
            for k in [0, 1, 3, 4, 6, 7, 2, 8, 5]:
                di, dj = k // 3, k % 3
                fp = fpsum.tile([128, Nc], F32, tag="fp")
                for m in range(nmm):
                    rr = r0 + 1 + m * rpm
                    nc.tensor.matmul(
                        fp[:, m * mm_cols : (m + 1) * mm_cols],
                        WT[:, k * 128 : (k + 1) * 128],
                        x3[:, rr : rr + rpm, 1 : W + 1],
                        start=True,
                        stop=True,
                    )
                fp3 = fp[:].rearrange("p (a b) -> p a b", a=CH)
                if k in EXT:
                    ei = EXT[k]
                    nc.scalar.activation(
                        FB[:, ei * Nc : (ei + 1) * Nc],
                        fp[:],
                        Identity,
                        bias=BIAS[:, k : k + 1],
                    )
                else:
                    P = pp.tile([128, Nc], F16, tag=f"p{k}")
                    P3 = P[:].rearrange("p (a b) -> p a b", a=CH)
                    nc.vector.scalar_tensor_tensor(
                        P3, fp3, BIAS[:, k : k + 1], win(r0 + di, dj, None, CH),
                        add, mult,
                    )
                    Tp[k] = P

            # products for the extracted taps: one 4D DVE op per tap trio
            PT = ptp.tile([128, 6 * Nc], F16, tag="pt")
            PT4 = PT[:].rearrange("p (e a b) -> p e a b", e=6, a=CH)
            nc.vector.tensor_tensor(
                PT4[:, 0:3], FB4[:, 0:3], win(r0, 0, 3, CH), mult
            )
            nc.vector.tensor_tensor(
                PT4[:, 3:6], FB4[:, 3:6], win(r0, 2, 3, CH), mult
            )
            # GpSimd folds the three STT product tiles into one
            D1 = dp.tile([128, Nc], F16, tag="d1")
            nc.gpsimd.tensor_tensor(D1[:], Tp[1][:], Tp[4][:], add)
            D2 = dp.tile([128, Nc], F16, tag="d2")
            nc.gpsimd.tensor_tensor(D2[:], D1[:], Tp[7][:], add)

            # PE identity-fold of the previous chunk (so the PE never waits
            # on this chunk's products): accumulate 7 tiles into out psum
            if prev_fold is not None:
                emit_fold(nc, tc, opsum, outp, out_ext, IDT, prev_fold, H, W, CH,
                          mm_cols)
            prev_fold = ([PT[:, i * Nc : (i + 1) * Nc] for i in range(6)] + [D2],
                         r0)

        emit_fold(nc, tc, opsum, outp, out_ext, IDT, prev_fold, H, W, CH, mm_cols)

    nc.compile()
    return nc


def emit_fold(nc, tc, opsum, outp, out_ext, IDT, fold, H, W, CH, mm_cols):
    tiles, r0 = fold
    Nc = CH * W
    op = opsum.tile([128, Nc], F32, tag="op")
    for fi, t in enumerate(tiles):
        for m in range(Nc // mm_cols):
            sl = slice(m * mm_cols, (m + 1) * mm_cols)
            nc.tensor.matmul(
                op[:, sl],
                IDT[:],
                t[:, sl],
                start=(fi == 0),
                stop=(fi == len(tiles) - 1),
            )
    OUT = outp.tile([128, Nc], F32, tag="out")
    nc.scalar.copy(OUT[:], op[:])
    o3 = OUT[:].rearrange("p (a b) -> p a b", a=CH)
    nc.sync.dma_start(out_ext[:, r0 : r0 + CH, :], o3[0:64])
    nc.sync.dma_start(out_ext[:, H // 2 + r0 : H // 2 + r0 + CH, :], o3[64:128])


_NC_CACHE = {}


def _get_nc(H, W, CH):
    key = (H, W, CH)
    if key not in _NC_CACHE:
        _NC_CACHE[key] = build_nc(H, W, CH)
    return _NC_CACHE[key]


def run(x, w_gen, b_gen, trace=False, tmpdir=None):
    x = np.asarray(x, dtype=np.float32)
    w_gen = np.asarray(w_gen, dtype=np.float32)
    b_gen = np.asarray(b_gen, dtype=np.float32)
    B, c, H, W = x.shape
    assert c == C and B == N_CORES

    wt, bias, idt = pack_weights(w_gen, b_gen)
    nc = _get_nc(H, W, 8)

    in_maps = [
        {"x": np.ascontiguousarray(x[i]), "wt": wt, "bias": bias, "idt": idt}
        for i in range(B)
    ]
    res = run_bass_kernel_spmd(
        nc, in_maps, core_ids=list(range(N_CORES)), trace=trace, tmpdir=tmpdir
    )
    out = np.stack([res.results[i]["out"] for i in range(B)], axis=0)
    return out, res


def kernel(x: np.ndarray, w_gen: np.ndarray, b_gen: np.ndarray) -> np.ndarray:
    return run(x, w_gen, b_gen)[0]


# revision 14
# speedup vs baseline: 1.1793x; 1.0315x over previous
import sys
from contextlib import ExitStack

import numpy as np

sys.path.insert(0, "/opt/trn_rl_repo")

import concourse.bass as bass  # noqa: E402
import concourse.mybir as mybir  # noqa: E402
import concourse.tile as tile  # noqa: E402
from concourse import bacc  # noqa: E402
from concourse.bass_utils import run_bass_kernel_spmd  # noqa: E402

C = 64
N_CORES = 8

F16 = mybir.dt.float16
F32 = mybir.dt.float32

# Half-image layout: partitions 0:64 = channels of image rows 0..63,
# partitions 64:128 = channels of rows 64..127.  A single padded fp16 plane
# [128, (H/2+2)*(W+2)] serves every tap as a plain offset view (identical
# offset for both halves), so no pre-shifted staging copies are needed and
# the output needs no cross-partition fold.
#
# Per tap k=(di,dj): f_k = blockdiag(Wk^T, Wk^T) @ x  (K=128 matmul, both
# halves at once), then T_k = (f_k + b_k) * patch_k elementwise, and
# out = sum_k T_k.
#
# Product routing (per 9 taps):
#   EXT  (dj=0 trio + dj=2 pair): ScalarE extracts f+b to fp16, DVE
#        multiplies 3/2 taps per op via 4D strided window APs (2x fp16 rate)
#   SDVE (tap 8): DVE scalar_tensor_tensor straight from PSUM
#   POOL (dj=1 trio): GpSimd scalar_tensor_tensor straight from PSUM
# Fold: DVE pairwise adds reduce the 5 extracted products to one tile;
# PE identity matmuls accumulate that tile + the 4 PSUM-route tiles into
# the output psum, which ScalarE copies out for the store DMA.

EXT = {0: 0, 3: 1, 6: 2, 2: 3, 5: 4, 8: 5}  # tap -> FB slot; dj=0 and dj=2 trios
SDVE = [1, 4, 7]  # dj=1 trio: DVE STT straight from PSUM (GpSimd can't read PSUM)


def pack_weights(w_gen: np.ndarray, b_gen: np.ndarray):
    W3 = w_gen.reshape(C, 9, C).astype(np.float32)  # [c, k, c']
    b3 = b_gen.reshape(C, 9).astype(np.float32)
    wt = np.zeros((128, 9 * 128), np.float32)
    bias = np.zeros((128, 9), np.float32)
    for k in range(9):
        blk = W3[:, k, :].T  # [c', c]
        wt[0:C, k * 128 : k * 128 + C] = blk
        wt[C:128, k * 128 + C : k * 128 + 128] = blk
        bias[0:C, k] = b3[:, k]
        bias[C:128, k] = b3[:, k]
    idt = np.eye(128, dtype=np.float32)
    return wt.astype(np.float16), bias.astype(np.float32), idt.astype(np.float16)


def build_nc(H=128, W=128, CH=8):
    HH = H // 2  # rows per half
    S = W + 2  # padded row stride
    PR = HH + 2  # padded rows per half-plane
    PS = PR * S
    Nc = CH * W  # pixels per chunk (per half)
    nch = HH // CH
    mm_cols = 512  # psum-bank limit for fp32 matmul output
    rpm = mm_cols // W
    nmm = Nc // mm_cols

    nc = bacc.Bacc("TRN2", target_bir_lowering=False)
    x_in = nc.declare_dram_parameter("x", [C, H, W], F32, isOutput=False)
    wt_in = nc.declare_dram_parameter("wt", [128, 9 * 128], F16, isOutput=False)
    bias_in = nc.declare_dram_parameter("bias", [128, 9], F32, isOutput=False)
    idt_in = nc.declare_dram_parameter("idt", [128, 128], F16, isOutput=False)
    out_ext = nc.declare_dram_parameter("out", [C, H, W], F32, isOutput=True)

    add = mybir.AluOpType.add
    mult = mybir.AluOpType.mult
    Identity = mybir.ActivationFunctionType.Identity

    with ExitStack() as ctx:
        tc = ctx.enter_context(tile.TileContext(nc))
        const = ctx.enter_context(tc.tile_pool(name="const", bufs=1))
        fpsum = ctx.enter_context(tc.tile_pool(name="fpsum", bufs=3, space="PSUM"))
        opsum = ctx.enter_context(tc.tile_pool(name="opsum", bufs=1, space="PSUM"))
        fbp = ctx.enter_context(tc.tile_pool(name="fbp", bufs=2))
        ptp = ctx.enter_context(tc.tile_pool(name="ptp", bufs=2))
        pp = ctx.enter_context(tc.tile_pool(name="pp", bufs=6))
        dp = ctx.enter_context(tc.tile_pool(name="dp", bufs=4))
        outp = ctx.enter_context(tc.tile_pool(name="outp", bufs=2))

        # +S tail margin: padded-flat product reads overrun the last plane row
        X = const.tile([128, PS + S], F16)
        WT = const.tile([128, 9 * 128], F16)
        BIAS = const.tile([128, 9], F32)
        IDT = const.tile([128, 128], F16)

        nc.sync.dma_start(WT[:], wt_in[:])
        nc.sync.dma_start(BIAS[:], bias_in[:])
        nc.sync.dma_start(IDT[:], idt_in[:])

        x3 = X[:, 0:PS].rearrange("p (h w) -> p h w", h=PR)

        # zero the borders: top halo row of the top half, bottom halo row of
        # the bottom half, and the left/right pad columns everywhere
        nc.vector.memset(x3[0:C, 0, :], 0.0)
        nc.vector.memset(x3[C:128, PR - 1, :], 0.0)
        nc.gpsimd.memset(x3[:, :, 0], 0.0)
        nc.gpsimd.memset(x3[:, :, W + 1], 0.0)

        # banded cast-loads (fp32 -> fp16): plane rows 1..65 of the top half
        # are image rows 0..64; plane rows 0..64 of the bottom half are image
        # rows 63..127.  Early bands are small so chunk 0 starts quickly.
        bands = [0, CH + 2] + [CH + 2 + CH * i for i in range(1, nch - 1)] + [PR - 1]
        for b in range(len(bands) - 1):
            r0, r1 = bands[b], bands[b + 1]
            nc.gpsimd.dma_start(
                out=x3[0:C, 1 + r0 : 1 + r1, 1 : W + 1],
                in_=x_in[:, r0:r1, :],
            )
            nc.gpsimd.dma_start(
                out=x3[C:128, r0:r1, 1 : W + 1],
                in_=x_in[:, HH - 1 + r0 : HH - 1 + r1, :],
            )

        CHS = CH * S

        def win_flat(base_row, dj, count):
            """[128, count, CH*S] trio window: per tap one contiguous stretch
            over the padded plane (pad columns ride along as junk)."""
            base = x3[:, base_row : base_row + 1, dj : dj + 1]
            w = base.copy()
            w.ap = mybir.VecI64Pair([tuple(w.ap[0]), (S, count), (1, CHS)])
            return w

        prev_fold = None  # (rhs AP lists, r0) from the previous chunk
        for n in range(nch):
            r0 = n * CH
            # FB/PT share the padded-plane row stride so the trio products
            # run as one contiguous stretch per tap (full 2x fp16 DVE rate)
            FB = fbp.tile([128, 6 * CHS], F16, tag="fb")
            FB3 = FB[:].rearrange("p (e h w) -> p e h w", e=6, w=S)
            FBf = FB[:].rearrange("p (e q) -> p e q", q=CHS)
            Tp = [None] * 9
            for k in [0, 1, 3, 4, 6, 7, 2, 8, 5]:
                di, dj = k // 3, k % 3
                fp = fpsum.tile([128, Nc], F32, tag="fp")
                for m in range(nmm):
                    rr = r0 + 1 + m * rpm
                    nc.tensor.matmul(
                        fp[:, m * mm_cols : (m + 1) * mm_cols],
                        WT[:, k * 128 : (k + 1) * 128],
                        x3[:, rr : rr + rpm, 1 : W + 1],
                        start=True,
                        stop=True,
                    )
                fp3 = fp[:].rearrange("p (a b) -> p a b", a=CH)
                if k in EXT:
                    ei = EXT[k]
                    nc.scalar.activation(
                        FB3[:, ei, :, 0:W],
                        fp3,
                        Identity,
                        bias=BIAS[:, k : k + 1],
                    )
                else:
                    P = pp.tile([128, Nc], F16, tag=f"p{k}")
                    P3 = P[:].rearrange("p (a b) -> p a b", a=CH)
                    nc.vector.scalar_tensor_tensor(
                        P3, fp3, BIAS[:, k : k + 1],
                        x3[:, r0 + di : r0 + di + CH, dj : dj + W],
                        add, mult,
                    )
                    Tp[k] = P

            # products for the extracted taps: one flat DVE op per tap trio
            PT = ptp.tile([128, 6 * CHS], F16, tag="pt")
            PT3 = PT[:].rearrange("p (e h w) -> p e h w", e=6, w=S)
            PTf = PT[:].rearrange("p (e q) -> p e q", q=CHS)
            nc.vector.tensor_tensor(
                PTf[:, 0:3], FBf[:, 0:3], win_flat(r0, 0, 3), mult
            )
            nc.vector.tensor_tensor(
                PTf[:, 3:6], FBf[:, 3:6], win_flat(r0, 2, 3), mult
            )
            # GpSimd folds two of the STT product tiles into one
            D1 = dp.tile([128, Nc], F16, tag="d1")
            nc.gpsimd.tensor_tensor(D1[:], Tp[1][:], Tp[4][:], add)

            # PE identity-fold of the previous chunk (so the PE never waits
            # on this chunk's products): accumulate 8 tiles into out psum
            if prev_fold is not None:
                emit_fold(nc, tc, opsum, outp, out_ext, IDT, prev_fold, H, W, CH,
                          mm_cols)
            rhs = [
                [PT3[:, i, m * rpm : (m + 1) * rpm, 0:W] for m in range(nmm)]
                for i in range(6)
            ] + [
                [t[:, m * mm_cols : (m + 1) * mm_cols] for m in range(nmm)]
                for t in (Tp[7], D1)
            ]
            prev_fold = (rhs, r0)

        emit_fold(nc, tc, opsum, outp, out_ext, IDT, prev_fold, H, W, CH, mm_cols)

    nc.compile()
    return nc


def emit_fold(nc, tc, opsum, outp, out_ext, IDT, fold, H, W, CH, mm_cols):
    rhs, r0 = fold
    Nc = CH * W
    op = opsum.tile([128, Nc], F32, tag="op")
    for fi, aps in enumerate(rhs):
        for m, ap in enumerate(aps):
            sl = slice(m * mm_cols, (m + 1) * mm_cols)
            nc.tensor.matmul(
                op[:, sl],
                IDT[:],
                ap,
                start=(fi == 0),
                stop=(fi == len(rhs) - 1),
            )
    OUT = outp.tile([128, Nc], F32, tag="out")
    nc.scalar.copy(OUT[:], op[:])
    o3 = OUT[:].rearrange("p (a b) -> p a b", a=CH)
    nc.sync.dma_start(out_ext[:, r0 : r0 + CH, :], o3[0:64])
    nc.sync.dma_start(out_ext[:, H // 2 + r0 : H // 2 + r0 + CH, :], o3[64:128])


_NC_CACHE = {}


def _get_nc(H, W, CH):
    key = (H, W, CH)
    if key not in _NC_CACHE:
        _NC_CACHE[key] = build_nc(H, W, CH)
    return _NC_CACHE[key]


def run(x, w_gen, b_gen, trace=False, tmpdir=None):
    x = np.asarray(x, dtype=np.float32)
    w_gen = np.asarray(w_gen, dtype=np.float32)
    b_gen = np.asarray(b_gen, dtype=np.float32)
    B, c, H, W = x.shape
    assert c == C and B == N_CORES

    wt, bias, idt = pack_weights(w_gen, b_gen)
    nc = _get_nc(H, W, 8)

    in_maps = [
        {"x": np.ascontiguousarray(x[i]), "wt": wt, "bias": bias, "idt": idt}
        for i in range(B)
    ]
    res = run_bass_kernel_spmd(
        nc, in_maps, core_ids=list(range(N_CORES)), trace=trace, tmpdir=tmpdir
    )
    out = np.stack([res.results[i]["out"] for i in range(B)], axis=0)
    return out, res


def kernel(x: np.ndarray, w_gen: np.ndarray, b_gen: np.ndarray) -> np.ndarray:
    return run(x, w_gen, b_gen)[0]
